# revision 25
# baseline (speedup 1.0000x reference)
"""GCN(2-layer) + single-step BiLSTM + MLP over a 100k-node/1.6M-edge graph,
distributed over 8 Trainium2 NeuronCores (Bass/Tile kernel).

v2 strategy (JIT-specialized to the input graph):
  - Node->(core,slot) assignment is DEGREE-BALANCED (greedy largest-first
    over the 784 dst blocks) so the SPMD structural-max tile layout has
    ~10% padding instead of ~43%.
  - Layer 1: per-edge message stream (dinv[s]*dinv[d]*x[s], bf16) host-
    gathered in edge order, streamed with big contiguous DMAs. Full norm
    is baked into the stream so the L1 drain has no per-column multiply.
  - One-hot scatter tiles are generated on-chip (DVE is_equal vs iota),
    with a fraction offloaded to the idle GpSimd engine during L1.
  - gs2 (layer-2 source table) uses a SLICE-MAJOR layout: rows grouped by
    (source superblock, core). Each slice is AllGather'd right after its
    L1 drain, and each slice is exactly one int16 gather window.
  - Layer 2: pull-gather of source rows with dma_gather (bf16, int16
    window-local indices) round-robin over 4 SWDGE queues. PAD slots use
    STRIPED indices (not row 0) — a constant pad row turns one DRAM
    channel into a hotspot and halves gather throughput.
  - PSUM accumulation uses start/stop per block-column (no memsets), and
    the tail (BiLSTM+MLP) runs per superblock right after its drain,
    entirely from SBUF.
"""
import sys

sys.path.insert(0, '/opt/trn_rl_repo')

import numpy as np
import ml_dtypes

N = 100000
F = 128
NCORES = 8
NPC = 12500           # real dst nodes per core
NSLOT = 12544         # 98 blocks of 128 slots
SBN = 3584            # slots per superblock (28 blocks); sb3 has 1792 (14)
NSB = 4
S_SZ = [3584, 3584, 3584, 1792]       # slots per sb (= source slice sizes)
SB_BASE = [0, 3584, 7168, 10752]
W_SZ = [8 * s for s in S_SZ]          # gather window sizes (<=28672 < 2^15)
SLICE_BASE = [0, 28672, 57344, 86016]
GS_ROWS = 100352                      # 8 * 12544, slice-major
NBLK_SB = [28, 28, 28, 14]
TILE = 128
CHUNK_TILES = 32      # L1 stream chunk
CHUNK2 = 16           # L2 gather chunk (2048 idx)
NQ = 4                # SWDGE queues
POOL_SGEN_L1 = 0      # GpSimd lacks TENSOR_TENSOR is_equal (ISA check); off

bf16 = ml_dtypes.bfloat16


def _balanced_assign(deg):
    """Assign nodes to (core, slot), balancing block loads by in-degree.

    Returns (core_of, slot_of) arrays [N]."""
    import heapq
    nblocks = NCORES * 98
    cap = np.full(nblocks, 128, dtype=np.int64)
    cap[97::98] = 84          # last block of each core holds 84 real nodes
    order = np.argsort(-deg, kind="stable")
    heap = [(0, b) for b in range(nblocks)]
    heapq.heapify(heap)
    members = [[] for _ in range(nblocks)]
    loads = np.zeros(nblocks, dtype=np.int64)
    nfil = np.zeros(nblocks, dtype=np.int64)
    for n in order:
        d = int(deg[n])
        while True:
            l, b = heapq.heappop(heap)
            if nfil[b] < cap[b]:
                break
        members[b].append(n)
        nfil[b] += 1
        loads[b] += d
        if nfil[b] < cap[b]:
            heapq.heappush(heap, (loads[b], b))
    core_of = np.zeros(N, dtype=np.int64)
    slot_of = np.zeros(N, dtype=np.int64)
    for b in range(nblocks):
        c, bi = b // 98, b % 98
        base = bi * 128
        mem = np.asarray(members[b], dtype=np.int64)
        core_of[mem] = c
        slot_of[mem] = base + np.arange(len(mem))
    return core_of, slot_of


def _chunklist(ntiles, step):
    out, t0 = [], 0
    while t0 < ntiles:
        n = min(step, ntiles - t0)
        out.append((t0, n))
        t0 += n
    return out


def prep(edge_index):
    """Host-side graph preprocessing -> (meta, per_core)."""
    src_in = np.asarray(edge_index[0], dtype=np.int64)
    dst_in = np.asarray(edge_index[1], dtype=np.int64)
    loops = np.arange(N, dtype=np.int64)
    src = np.concatenate([src_in, loops])
    dst = np.concatenate([dst_in, loops])

    deg = np.bincount(dst, minlength=N).astype(np.float64)
    dinv = (np.maximum(deg, 1e-12) ** -0.5).astype(np.float32)

    core_of, slot_of = _balanced_assign(deg.astype(np.int64))

    slot_s = slot_of[src]
    w_all = np.minimum(slot_s // SBN, NSB - 1)
    gs_row_all = (np.asarray(SLICE_BASE)[w_all] + core_of[src] * np.asarray(S_SZ)[w_all]
                  + (slot_s - np.asarray(SB_BASE)[w_all]))
    dst_core = core_of[dst]
    dst_slot = slot_of[dst]

    # ---------------- per-core edge partitions ----------------
    cores = []
    sizes1 = np.zeros((NCORES, NSB, 28), dtype=np.int64)
    sizes2 = np.zeros((NCORES, NSB, NSB, 28), dtype=np.int64)
    for c in range(NCORES):
        m = dst_core == c
        s_c = src[m]
        dl = dst_slot[m]
        sb = dl // SBN
        bi = (dl - sb * SBN) // 128
        lane = dl % 128
        w = w_all[m]
        o1 = np.lexsort((s_c, dl))
        o2 = np.lexsort((s_c, dl, bi, w, sb))
        d = {"s1": s_c[o1], "sb1": sb[o1], "bi1": bi[o1], "lane1": lane[o1],
             "s2": s_c[o2], "sb2": sb[o2], "bi2": bi[o2], "lane2": lane[o2],
             "w2": w[o2], "gs2": gs_row_all[m][o2]}
        cores.append(d)
        k1 = d["sb1"] * 28 + d["bi1"]
        sizes1[c] = np.bincount(k1, minlength=NSB * 28).reshape(NSB, 28)
        k2 = (d["sb2"] * NSB + d["w2"]) * 28 + d["bi2"]
        sizes2[c] = np.bincount(k2, minlength=NSB * NSB * 28).reshape(NSB, NSB, 28)

    # ---------------- structural tile layout: L1 ----------------
    T1 = -(-sizes1.max(axis=0) // TILE)           # [NSB, 28]
    for sb in range(NSB):
        T1[sb, :NBLK_SB[sb]] = np.maximum(1, T1[sb, :NBLK_SB[sb]])
        T1[sb, NBLK_SB[sb]:] = 0
    tbase1 = np.zeros((NSB, 28), dtype=np.int64)
    Tsb1 = np.zeros(NSB, dtype=np.int64)
    for sb in range(NSB):
        off = 0
        for b in range(NBLK_SB[sb]):
            tbase1[sb, b] = off
            off += T1[sb, b]
        Tsb1[sb] = off
    sbbase1 = np.zeros(NSB + 1, dtype=np.int64)
    sbbase1[1:] = np.cumsum(Tsb1)
    T1TOT = int(sbbase1[-1])

    tinfo1 = {}
    for sb in range(NSB):
        nt = int(Tsb1[sb])
        blk = np.zeros(nt, dtype=np.int64)
        first = np.zeros(nt, dtype=bool)
        last = np.zeros(nt, dtype=bool)
        for b in range(NBLK_SB[sb]):
            a, n = int(tbase1[sb, b]), int(T1[sb, b])
            blk[a:a + n] = b
            first[a] = True
            last[a + n - 1] = True
        tinfo1[sb] = (blk, first, last)
    chunks1 = {sb: _chunklist(int(Tsb1[sb]), CHUNK_TILES) for sb in range(NSB)}

    # ---------------- structural tile layout: L2 ----------------
    T2 = np.maximum(1, -(-sizes2.max(axis=0) // TILE))  # [NSB, NSB(w), 28]
    for sb in range(NSB):
        T2[sb, :, NBLK_SB[sb]:] = 0
    tbase2 = np.zeros((NSB, NSB, 28), dtype=np.int64)
    Tsw2 = np.zeros((NSB, NSB), dtype=np.int64)
    for sb in range(NSB):
        for w in range(NSB):
            off = 0
            for b in range(NBLK_SB[sb]):
                tbase2[sb, w, b] = off
                off += T2[sb, w, b]
            Tsw2[sb, w] = off
    swbase2 = np.zeros((NSB, NSB), dtype=np.int64)
    off = 0
    for sb in range(NSB):
        for w in range(NSB):
            swbase2[sb, w] = off
            off += int(Tsw2[sb, w])
    T2TOT = int(off)

    tinfo2 = {}
    for sb in range(NSB):
        for w in range(NSB):
            nt = int(Tsw2[sb, w])
            blk = np.zeros(nt, dtype=np.int64)
            first = np.zeros(nt, dtype=bool)
            last = np.zeros(nt, dtype=bool)
            for b in range(NBLK_SB[sb]):
                a, n = int(tbase2[sb, w, b]), int(T2[sb, w, b])
                blk[a:a + n] = b
                if w == 0:
                    first[a] = True
                if w == NSB - 1:
                    last[a + n - 1] = True
            tinfo2[(sb, w)] = (blk, first, last)
    chunks2 = {(sb, w): _chunklist(int(Tsw2[sb, w]), CHUNK2)
               for sb in range(NSB) for w in range(NSB)}

    # ---------------- per-core padded streams ----------------
    per_core = []
    for c in range(NCORES):
        d = cores[c]

        # --- L1: host-gather permutation + lane stream ---
        ne = len(d["s1"])
        k1 = d["sb1"] * 28 + d["bi1"]
        cnt = np.bincount(k1, minlength=NSB * 28)
        g0 = np.zeros(NSB * 28 + 1, dtype=np.int64)
        g0[1:] = np.cumsum(cnt)
        rank = np.arange(ne) - g0[k1]
        pos1 = (sbbase1[d["sb1"]] + tbase1[d["sb1"], d["bi1"]]) * TILE + rank

        sperm = np.zeros(T1TOT * TILE, dtype=np.int64)
        sval = np.zeros(T1TOT * TILE, dtype=np.float32)
        dstloc1 = np.full(T1TOT * TILE, 255.0, dtype=np.float32)
        sperm[pos1] = d["s1"]
        dl_full = d["sb1"] * SBN + d["bi1"] * 128 + d["lane1"]
        sval[pos1] = dinv[d["s1"]] * dinv_of_slot(dinv, core_of, slot_of, c, dl_full)
        dstloc1[pos1] = d["lane1"].astype(np.float32)
        dl1_t = dstloc1.reshape(-1, TILE).T

        # --- L2: gather indices (striped pads) + lane stream ---
        ne2 = len(d["s2"])
        k2 = (d["sb2"] * NSB + d["w2"]) * 28 + d["bi2"]
        cnt2 = np.bincount(k2, minlength=NSB * NSB * 28)
        g20 = np.zeros(NSB * NSB * 28 + 1, dtype=np.int64)
        g20[1:] = np.cumsum(cnt2)
        rank2 = np.arange(ne2) - g20[k2]
        pos2 = ((swbase2[d["sb2"], d["w2"]]
                 + tbase2[d["sb2"], d["w2"], d["bi2"]]) * TILE + rank2)

        # striped pad indices: spread pad reads over the whole window
        idxg = np.zeros(T2TOT * TILE, dtype=np.int16)
        for sb in range(NSB):
            for w in range(NSB):
                t0 = int(swbase2[sb, w]) * TILE
                t1 = t0 + int(Tsw2[sb, w]) * TILE
                p = np.arange(t0, t1, dtype=np.int64)
                idxg[t0:t1] = ((p * 97) % W_SZ[w]).astype(np.int16)
        dstloc2 = np.full(T2TOT * TILE, 255.0, dtype=np.float32)
        idxg[pos2] = (d["gs2"] - np.asarray(SLICE_BASE)[d["w2"]]).astype(np.int16)
        dstloc2[pos2] = d["lane2"].astype(np.float32)
        idx_pack = np.tile(idxg.reshape(-1, 16).T, (8, 1))
        dl2_t = dstloc2.reshape(-1, TILE).T

        per_core.append({
            "sperm": sperm,
            "sval": sval,
            "dl1": np.ascontiguousarray(dl1_t.astype(bf16)),
            "idxg": np.ascontiguousarray(idx_pack),
            "dl2": np.ascontiguousarray(dl2_t.astype(bf16)),
        })

    Tsb2 = Tsw2.sum(axis=1)
    meta = {"T1TOT": T1TOT, "T2TOT": T2TOT, "Tsb1": Tsb1, "Tsw2": Tsw2,
            "Tsb2": Tsb2, "sbbase1": sbbase1, "swbase2": swbase2,
            "tinfo1": tinfo1, "tinfo2": tinfo2,
            "chunks1": chunks1, "chunks2": chunks2,
            "dinv": dinv, "core_of": core_of, "slot_of": slot_of}
    return meta, per_core


def dinv_of_slot(dinv, core_of, slot_of, c, slots):
    """dinv value of the node occupying (core c, slot) — via inverse map."""
    # build once per call site; cached by caller pattern (small)
    inv = np.zeros(NSLOT, dtype=np.float32)
    mask = core_of == c
    inv[slot_of[mask]] = dinv[mask]
    return inv[slots]


# ---------------------------------------------------------------------------
# device program
# ---------------------------------------------------------------------------

def build_nc(meta):
    import concourse.bacc as bacc
    import concourse.mybir as mybir
    import concourse.tile as tile
    from concourse.masks import make_identity

    dt = mybir.dt
    T1TOT = meta["T1TOT"]
    T2TOT = meta["T2TOT"]
    Tsw2 = meta["Tsw2"]
    Tsb2 = meta["Tsb2"]
    sbbase1 = meta["sbbase1"]
    swbase2 = meta["swbase2"]
    tinfo1 = meta["tinfo1"]
    tinfo2 = meta["tinfo2"]
    chunks1 = meta["chunks1"]
    chunks2 = meta["chunks2"]
    maxTsb2 = int(Tsb2.max())

    nc = bacc.Bacc("TRN2", target_bir_lowering=False, debug=False,
                   num_devices=NCORES, num_swdge_queues=NQ)

    # ---- I/O ----
    xe_in = nc.dram_tensor("xe", [128, T1TOT, F], dt.bfloat16, kind="ExternalInput")
    dl1_in = nc.dram_tensor("dl1", [128, T1TOT], dt.bfloat16, kind="ExternalInput")
    idxg = nc.dram_tensor("idxg", [128, T2TOT * 8], dt.int16, kind="ExternalInput")
    dl2_in = nc.dram_tensor("dl2", [128, T2TOT], dt.bfloat16, kind="ExternalInput")
    iota_in = nc.dram_tensor("iotaf", [128, 128], dt.bfloat16, kind="ExternalInput")
    iotaw_in = nc.dram_tensor("iotaw", [128, 128 * CHUNK_TILES], dt.bfloat16,
                              kind="ExternalInput")
    drow = nc.dram_tensor("drow", [128, NSLOT], dt.float32, kind="ExternalInput")
    w1_in = nc.dram_tensor("w1", [128, 128], dt.bfloat16, kind="ExternalInput")
    w2_in = nc.dram_tensor("w2", [128, 128], dt.bfloat16, kind="ExternalInput")
    b1_in = nc.dram_tensor("b1c", [128, 1], dt.float32, kind="ExternalInput")
    b2_in = nc.dram_tensor("b2c", [128, 1], dt.float32, kind="ExternalInput")
    wihf_in = nc.dram_tensor("wihfT", [128, 512], dt.bfloat16, kind="ExternalInput")
    wihb_in = nc.dram_tensor("wihbT", [128, 512], dt.bfloat16, kind="ExternalInput")
    bsf_in = nc.dram_tensor("bsumf", [128, 4], dt.float32, kind="ExternalInput")
    bsb_in = nc.dram_tensor("bsumb", [128, 4], dt.float32, kind="ExternalInput")
    fc1_in = nc.dram_tensor("fc1", [128, 128], dt.bfloat16, kind="ExternalInput")
    fcb1_in = nc.dram_tensor("fcb1c", [64, 1], dt.float32, kind="ExternalInput")
    fc2_in = nc.dram_tensor("fc2", [64, 1], dt.bfloat16, kind="ExternalInput")
    fcb2_in = nc.dram_tensor("fcb2c", [1, 1], dt.float32, kind="ExternalInput")
    y_out = nc.dram_tensor("y", [NSLOT, 1], dt.float32, kind="ExternalOutput")


    gs2_in = nc.dram_tensor("gs2in", [NSLOT, F], dt.bfloat16, kind="Internal")
    gs2 = nc.dram_tensor("gs2", [GS_ROWS, F], dt.bfloat16, kind="Internal",
                         addr_space="Shared")

    qctr = [0]

    with tile.TileContext(nc) as tc:
        with tc.tile_pool(name="const", bufs=1) as cpool, \
             tc.tile_pool(name="idx", bufs=2) as ipool, \
             tc.tile_pool(name="msg", bufs=3) as mpool, \
             tc.tile_pool(name="sg", bufs=3) as spool, \
             tc.tile_pool(name="msg2", bufs=8) as mp2, \
             tc.tile_pool(name="sg2", bufs=4) as sp2, \
             tc.tile_pool(name="dr", bufs=1) as drpool, \
             tc.tile_pool(name="staged", bufs=2) as stpool, \
             tc.tile_pool(name="h1", bufs=2) as h1pool, \
             tc.tile_pool(name="ndm", bufs=1) as ndmpool, \
             tc.tile_pool(name="tail", bufs=2) as tpool, \
             tc.tile_pool(name="psA", bufs=7, space="PSUM") as psA, \
             tc.tile_pool(name="psB", bufs=1, space="PSUM") as psB:

            # ---- constants ----
            def const_tile(shape, dtt, src_ap, cname):
                t = cpool.tile(shape, dtt, tag=cname, name=cname)
                nc.sync.dma_start(t[:], src_ap)
                return t

            w1_t = const_tile([128, 128], dt.bfloat16, w1_in[:], "c_w1")
            w2_t = const_tile([128, 128], dt.bfloat16, w2_in[:], "c_w2")
            b1_t = const_tile([128, 1], dt.float32, b1_in[:], "c_b1")
            b2_t = const_tile([128, 1], dt.float32, b2_in[:], "c_b2")
            wihf_t = const_tile([128, 512], dt.bfloat16, wihf_in[:], "c_wihf")
            wihb_t = const_tile([128, 512], dt.bfloat16, wihb_in[:], "c_wihb")
            bsf_t = const_tile([128, 4], dt.float32, bsf_in[:], "c_bsf")
            bsb_t = const_tile([128, 4], dt.float32, bsb_in[:], "c_bsb")
            fc1_t = const_tile([128, 128], dt.bfloat16, fc1_in[:], "c_fc1")
            fcb1_t = const_tile([64, 1], dt.float32, fcb1_in[:], "c_fcb1")
            fc2_t = const_tile([64, 1], dt.bfloat16, fc2_in[:], "c_fc2")
            fcb2_t = const_tile([1, 1], dt.float32, fcb2_in[:], "c_fcb2")
            ident_t = cpool.tile([128, 128], dt.bfloat16)
            make_identity(nc, ident_t[:])
            iota_t = const_tile([128, 128], dt.bfloat16, iota_in[:], "c_iota")
            dl1_t = const_tile([128, T1TOT], dt.bfloat16, dl1_in[:], "c_dl1")
            dl2_t = const_tile([128, T2TOT], dt.bfloat16, dl2_in[:], "c_dl2")
            # wide iota: iw[p, d, j] = d  (for the 2x-mode transposed one-hot)
            iw_t = const_tile([128, 128, CHUNK_TILES], dt.bfloat16,
                              iotaw_in[:].rearrange("p (d j) -> p d j",
                                                    j=CHUNK_TILES), "c_iw")

            def gen_S(eng, pool, tag, nt_cap, dl_tile, c0, ntiles):
                """Transposed one-hot: S[p, d, t] = (dstloc[p, c0+t] == d).

                All operands 2-byte, innermost stride 1 -> DVE 2x mode.
                Use a tile's slice st[:, :, t] (col stride nt_cap) as the
                stationary lhsT of the aggregation matmul."""
                st = pool.tile([128, 128, nt_cap], dt.bfloat16, tag=tag)
                eng.tensor_tensor(
                    st[:, :, :ntiles],
                    dl_tile[:, None, c0:c0 + ntiles].to_broadcast(
                        [128, 128, ntiles]),
                    iw_t[:, :, :ntiles],
                    mybir.AluOpType.is_equal)
                return st

            # ---------------- layer 1: host-gathered stream ----------------
            cchunk = [0]
            for sb in range(NSB):
                ncols = NBLK_SB[sb]
                nbank = (ncols + 3) // 4
                blk, first, last = tinfo1[sb]
                gbase = int(sbbase1[sb])
                aggs = [psA.tile([128, 512], dt.float32, tag="agg",
                                 name=f"agg1_sb{sb}_k{k}")
                        for k in range(nbank)]
                for a in aggs:
                    nc.vector.memset(a[:], 0.0)
                for (t0, ntiles) in chunks1[sb]:
                    mt = mpool.tile([128, CHUNK_TILES, F], dt.bfloat16, tag="msg")
                    nc.sync.dma_start(mt[:, :ntiles, :],
                                      xe_in[:, gbase + t0:gbase + t0 + ntiles, :])
                    cchunk[0] += 1
                    eng = (nc.gpsimd if (POOL_SGEN_L1 and
                                         cchunk[0] % POOL_SGEN_L1 == 0)
                           else nc.vector)
                    st = gen_S(eng, spool, "S", CHUNK_TILES, dl1_t,
                               gbase + t0, ntiles)
                    for t in range(ntiles):
                        b = int(blk[t0 + t])
                        bank, col = b // 4, b % 4
                        nc.tensor.matmul(
                            aggs[bank][:, col * 128:(col + 1) * 128],
                            lhsT=mt[:, t, :],
                            rhs=st[:, :, t],
                            start=False,
                            stop=bool(last[t0 + t]),
                            skip_group_check=True)
                # ---- drain + transform into gs2_in slab ----
                sb_base = SB_BASE[sb]
                width = ncols * 128
                dr = drpool.tile([128, 28 * 128], dt.float32, tag="drow")
                nc.sync.dma_start(dr[:, :width], drow[:, sb_base:sb_base + width])
                h1sb = h1pool.tile([128, 28 * 128], dt.bfloat16, tag="h1sb")
                ndm = ndmpool.tile([128, 28, 128], dt.bfloat16, tag="ndm")
                for k in range(nbank):
                    wcols = min(4, ncols - k * 4) * 128
                    staged = stpool.tile([128, 512], dt.bfloat16, tag="staged")
                    nc.vector.tensor_copy(staged[:, :wcols], aggs[k][:, :wcols])
                    ptx = psB.tile([128, 512], dt.float32, tag="tx")
                    nc.tensor.matmul(ptx[:, :wcols], lhsT=w1_t[:],
                                     rhs=staged[:, :wcols],
                                     start=True, stop=True)
                    nc.scalar.activation(
                        h1sb[:, k * 512:k * 512 + wcols], ptx[:, :wcols],
                        mybir.ActivationFunctionType.Relu, bias=b1_t[:])
                ch0 = 0
                while ch0 < width:
                    cw = min(512, width - ch0)
                    ptx = psB.tile([128, 512], dt.float32, tag="tx")
                    nc.tensor.matmul(ptx[:, :cw], lhsT=w2_t[:],
                                     rhs=h1sb[:, ch0:ch0 + cw],
                                     start=True, stop=True)
                    gsT = h1pool.tile([128, 512], dt.bfloat16, tag="gsT")
                    nc.vector.tensor_tensor(
                        gsT[:, :cw], ptx[:, :cw],
                        dr[:, ch0:ch0 + cw], mybir.AluOpType.mult)
                    for bb in range(cw // 128):
                        b = ch0 // 128 + bb
                        ptp = psB.tile([128, 512], dt.float32, tag="tx")
                        ptp_b = ptp[:].bitcast(dt.bfloat16)[:, :128]
                        nc.tensor.transpose(
                            ptp_b, gsT[:, bb * 128:(bb + 1) * 128], ident_t[:])
                        nc.vector.tensor_copy(ndm[:, b, :], ptp_b)
                    ch0 += cw
                nc.sync.dma_start(
                    gs2_in[sb_base:sb_base + width, :]
                    .rearrange("(c p) f -> p c f", p=128),
                    ndm[:, :ncols, :])
                # ---- sliced AllGather: ship this source slice now ----
                nc.gpsimd.collective_compute(
                    "AllGather", mybir.AluOpType.bypass,
                    replica_groups=[list(range(NCORES))],
                    ins=[gs2_in[sb_base:sb_base + width, :]],
                    outs=[gs2[SLICE_BASE[sb]:SLICE_BASE[sb] + 8 * width, :]])

            # ---------------- layer 2: SWDGE gather ----------------
            tcol = [0]
            for sb in range(NSB):
                ncols = NBLK_SB[sb]
                nbank = (ncols + 3) // 4
                aggs = [psA.tile([128, 512], dt.float32, tag="agg",
                                 name=f"agg2_sb{sb}_k{k}")
                        for k in range(nbank)]
                for a in aggs:
                    nc.vector.memset(a[:], 0.0)
                sb_tile0 = int(swbase2[sb, 0])
                nt_sb = int(Tsb2[sb])
                it = ipool.tile([128, maxTsb2 * 8], dt.int16, tag="idx")
                nc.sync.dma_start(it[:, :nt_sb * 8],
                                  idxg[:, sb_tile0 * 8:(sb_tile0 + nt_sb) * 8])
                for w in range(NSB):
                    blk, first, last = tinfo2[(sb, w)]
                    wt0 = int(swbase2[sb, w]) - sb_tile0
                    lo = SLICE_BASE[w]
                    src_win = gs2[lo:lo + W_SZ[w], :]
                    for (t0, ntiles) in chunks2[(sb, w)]:
                        nidx = ntiles * TILE
                        mt = mp2.tile([128, CHUNK2, F], dt.bfloat16, tag="msg2")
                        nc.gpsimd.dma_gather(
                            mt[:, :ntiles, :], src_win,
                            it[:, (wt0 + t0) * 8:(wt0 + t0) * 8 + nidx // 16],
                            nidx, nidx, F, single_packet=False,
                            queue_num=qctr[0] % NQ)
                        qctr[0] += 1
                        st = gen_S(nc.vector, sp2, "S2", CHUNK2, dl2_t,
                                   tcol[0], ntiles)
                        for t in range(ntiles):
                            b = int(blk[t0 + t])
                            bank, col = b // 4, b % 4
                            nc.tensor.matmul(
                                aggs[bank][:, col * 128:(col + 1) * 128],
                                lhsT=mt[:, t, :],
                                rhs=st[:, :, t],
                                start=False,
                                stop=bool(last[t0 + t]),
                                skip_group_check=True)
                        tcol[0] += ntiles
                # ---- drain: h2 = relu(psum * dinv_dst + b2) -> SBUF ----
                sb_base = SB_BASE[sb]
                width = ncols * 128
                dr = drpool.tile([128, 28 * 128], dt.float32, tag="drow")
                nc.sync.dma_start(dr[:, :width], drow[:, sb_base:sb_base + width])
                h2sb = h1pool.tile([128, 28 * 128], dt.bfloat16, tag="h1sb")
                for k in range(nbank):
                    wcols = min(4, ncols - k * 4) * 128
                    staged = stpool.tile([128, 512], dt.bfloat16, tag="staged")
                    nc.vector.tensor_tensor(
                        staged[:, :wcols],
                        aggs[k][:, :wcols],
                        dr[:, k * 512:k * 512 + wcols],
                        mybir.AluOpType.mult)
                    nc.scalar.activation(
                        h2sb[:, k * 512:k * 512 + wcols],
                        staged[:, :wcols],
                        mybir.ActivationFunctionType.Relu, bias=b2_t[:])
                # ---- tail for this superblock: BiLSTM step + MLP ----
                ch0 = 0
                while ch0 < width:
                    cw = min(512, width - ch0)
                    h2c = h2sb[:, ch0:ch0 + cw]
                    hdir = []
                    for (wih_t, bs_t) in ((wihf_t, bsf_t), (wihb_t, bsb_t)):
                        gates = {}
                        for jb, fn in ((0, "Sigmoid"), (2, "Tanh"),
                                       (3, "Sigmoid")):
                            pg = psB.tile([128, 512], dt.float32, tag="tx")
                            nc.tensor.matmul(
                                pg[:, :cw],
                                lhsT=wih_t[:, jb * 128:(jb + 1) * 128],
                                rhs=h2c, start=True, stop=True)
                            gt = tpool.tile([128, 512], dt.bfloat16,
                                            tag=f"g{jb}")
                            nc.scalar.activation(
                                gt[:, :cw], pg[:, :cw],
                                getattr(mybir.ActivationFunctionType, fn),
                                bias=bs_t[:, jb:jb + 1])
                            gates[jb] = gt
                        c_t = tpool.tile([128, 512], dt.bfloat16, tag="c")
                        nc.vector.tensor_tensor(
                            c_t[:, :cw], gates[0][:, :cw],
                            gates[2][:, :cw], mybir.AluOpType.mult)
                        tc_t = tpool.tile([128, 512], dt.bfloat16, tag="tc")
                        nc.scalar.activation(
                            tc_t[:, :cw], c_t[:, :cw],
                            mybir.ActivationFunctionType.Tanh)
                        h_t = tpool.tile([128, 512], dt.bfloat16,
                                         tag=f"h{len(hdir)}")
                        nc.vector.tensor_tensor(
                            h_t[:, :cw], gates[3][:, :cw],
                            tc_t[:, :cw], mybir.AluOpType.mult)
                        hdir.append(h_t)
                    py1 = psB.tile([128, 512], dt.float32, tag="tx")
                    nc.tensor.matmul(py1[:64, :cw], lhsT=fc1_t[:, :64],
                                     rhs=hdir[0][:, :cw], start=True, stop=False)
                    nc.tensor.matmul(py1[:64, :cw], lhsT=fc1_t[:, 64:],
                                     rhs=hdir[1][:, :cw], start=False, stop=True)
                    y1_t = tpool.tile([64, 512], dt.bfloat16, tag="y1")
                    nc.scalar.activation(y1_t[:, :cw], py1[:64, :cw],
                                         mybir.ActivationFunctionType.Relu,
                                         bias=fcb1_t[:])
                    py2 = psB.tile([128, 512], dt.float32, tag="tx")
                    nc.tensor.matmul(py2[:1, :cw], lhsT=fc2_t[:],
                                     rhs=y1_t[:, :cw], start=True, stop=True)
                    ych = tpool.tile([1, 512], dt.float32, tag="ych")
                    nc.vector.tensor_scalar_add(ych[0:1, :cw],
                                                py2[:1, :cw], fcb2_t[0:1, 0:1])
                    nc.sync.dma_start(
                        y_out[sb_base + ch0:sb_base + ch0 + cw, :]
                        .rearrange("n o -> o n"),
                        ych[0:1, :cw])
                    ch0 += cw

    nc.compile()
    return nc


_CACHE = {}


def _marshal(inputs, meta, per_core):
    x = np.asarray(inputs["x"], dtype=np.float32)
    dinv = meta["dinv"]
    core_of = meta["core_of"]
    slot_of = meta["slot_of"]
    T1TOT = meta["T1TOT"]
    iota = np.ascontiguousarray(
        np.broadcast_to(np.arange(128, dtype=np.float32), (128, 128)).astype(bf16))
    in_common = {
        "w1": np.ascontiguousarray(np.asarray(inputs["W1"], np.float32).astype(bf16)),
        "w2": np.ascontiguousarray(np.asarray(inputs["W2"], np.float32).astype(bf16)),
        "b1c": np.ascontiguousarray(np.asarray(inputs["b1"], np.float32)[:, None]),
        "b2c": np.ascontiguousarray(np.asarray(inputs["b2"], np.float32)[:, None]),
        "wihfT": np.ascontiguousarray(
            np.asarray(inputs["Wih_f"], np.float32).T.astype(bf16)),
        "wihbT": np.ascontiguousarray(
            np.asarray(inputs["Wih_b"], np.float32).T.astype(bf16)),
        "bsumf": np.ascontiguousarray(
            (np.asarray(inputs["bih_f"], np.float32)
             + np.asarray(inputs["bhh_f"], np.float32)).reshape(4, 128).T),
        "bsumb": np.ascontiguousarray(
            (np.asarray(inputs["bih_b"], np.float32)
             + np.asarray(inputs["bhh_b"], np.float32)).reshape(4, 128).T),
        "fc1": np.ascontiguousarray(
            np.asarray(inputs["fcW1"], np.float32).astype(bf16)
            .reshape(2, 128, 64).transpose(1, 0, 2).reshape(128, 128)),
        "fcb1c": np.ascontiguousarray(np.asarray(inputs["fcb1"], np.float32)[:, None]),
        "fc2": np.ascontiguousarray(np.asarray(inputs["fcW2"], np.float32).astype(bf16)),
        "fcb2c": np.ascontiguousarray(
            np.asarray(inputs["fcb2"], np.float32).reshape(1, 1)),
        "iotaf": iota,
        "iotaw": np.ascontiguousarray(np.broadcast_to(
            np.repeat(np.arange(128, dtype=np.float32), CHUNK_TILES)[None, :],
            (128, 128 * CHUNK_TILES)).astype(bf16)),
    }
    in_maps = []
    for c in range(NCORES):
        pc = per_core[c]
        drow_c = np.zeros((1, NSLOT), dtype=np.float32)
        mask = core_of == c
        drow_c[0, slot_of[mask]] = dinv[mask]
        drow_c = np.ascontiguousarray(np.tile(drow_c, (128, 1)))
        m = dict(in_common)
        m["drow"] = drow_c
        xg = (x[pc["sperm"]] * pc["sval"][:, None]).astype(bf16)
        m["xe"] = np.ascontiguousarray(
            xg.reshape(T1TOT, TILE, F).transpose(1, 0, 2))
        m["dl1"] = pc["dl1"]
        m["idxg"] = pc["idxg"]
        m["dl2"] = pc["dl2"]
        in_maps.append(m)
    return in_maps


def get_compiled(edge_index):
    import hashlib
    edge_index = np.asarray(edge_index)
    key = hashlib.sha1(edge_index.tobytes()).hexdigest()
    if key not in _CACHE:
        meta, per_core = prep(edge_index)
        nc = build_nc(meta)
        _CACHE[key] = (meta, per_core, nc)
    return _CACHE[key]


def run(inputs, trace=False):
    from concourse.bass_utils import run_bass_kernel_spmd
    meta, per_core, nc = get_compiled(inputs["edge_index"])
    in_maps = _marshal(inputs, meta, per_core)
    res = run_bass_kernel_spmd(nc, in_maps, core_ids=list(range(NCORES)),
                               trace=trace)
    core_of = meta["core_of"]
    slot_of = meta["slot_of"]
    y = np.zeros((N, 1), dtype=np.float32)
    for c in range(NCORES):
        yc = np.asarray(res.results[c]["y"])
        mask = core_of == c
        y[mask, 0] = yc[slot_of[mask], 0]
    return y, res


def kernel(**inputs):
    y, _ = run(inputs, trace=False)
    return y
